# revision 35
# baseline (speedup 1.0000x reference)
"""DGN (graph attention network) forward pass on 8 Trainium2 NeuronCores.

Strategy: pure data parallelism over the batch of 128 independent graphs
(16 graphs per core, weights replicated). Per graph, activations are kept
feature-major ([feature -> SBUF partitions, node -> free dim]) so weight
matrices serve directly as the stationary matmul operand. Attention is
computed k-major (scores^T[k, q]) so the softmax'd matrix feeds the AV
matmul without a transpose; softmax skips max-subtraction (scores are
tiny for this model family) and handles the mask via
  softmax(mask ? s : -inf) = mask*exp(s) / sum(mask*exp(s))
split as mask@V (head-shared) + ((exp(s)-1)*mask)@V for bf16 precision.
The denominator rides along as a ones-column appended to V.
"""

import os
import sys

for _p in ("/opt/trn_rl_repo",):
    if _p not in sys.path and os.path.isdir(_p):
        sys.path.append(_p)

import numpy as np

import concourse.bass as bass
import concourse.bacc as bacc
import concourse.tile as tile
from concourse import mybir
from concourse.masks import make_identity

F32 = mybir.dt.float32
BF16 = mybir.dt.bfloat16
I32 = mybir.dt.int32

B = 128          # total graphs
NCORES = 8
G = B // NCORES  # graphs per core
N = 256          # nodes per graph
NT = N // 128    # node tiles
F_IN = 128
HID = 512
KT = HID // 128  # K tiles over hidden dim
H = 8            # heads
D = 16           # head dim
HD = H * D       # 128
A = 32           # num actions
SCALE = 1.0 / (D ** 0.5)

WEIGHT_NAMES = [
    "enc_W1", "enc_b1", "enc_W2", "enc_b2",
    "Wv1", "bv1", "Wk1", "bk1", "Wq1", "bq1", "Wo1", "bo1",
    "Wv2", "bv2", "Wk2", "bk2", "Wq2", "bq2", "Wo2", "bo2",
    "q_W", "q_b",
]


def _emit(nc, tc, ap, g_count):
    """Emit the full per-core program. ap: dict name -> DRAM AP."""
    import contextlib
    ctx = contextlib.ExitStack()
    with ctx:
        # ---------------- pools ----------------
        wp = ctx.enter_context(tc.tile_pool(name="wp", bufs=1))       # persistent weights
        stg = ctx.enter_context(tc.tile_pool(name="stg", bufs=2))     # f32 weight staging
        gio = ctx.enter_context(tc.tile_pool(name="gio", bufs=6))     # per-graph dma-in tiles
        act = ctx.enter_context(tc.tile_pool(name="act", bufs=4))     # per-graph activations
        sml = ctx.enter_context(tc.tile_pool(name="sml", bufs=5))     # small per-use tiles
        esp = ctx.enter_context(tc.tile_pool(name="esp", bufs=2))     # exp tiles
        mep = ctx.enter_context(tc.tile_pool(name="mep", bufs=3))     # masked-exp tiles
        pmm = ctx.enter_context(tc.tile_pool(name="pmm", bufs=2, space="PSUM"))  # [128,2,256] f32
        psc = ctx.enter_context(tc.tile_pool(name="psc", bufs=1, space="PSUM"))  # scores
        pav = ctx.enter_context(tc.tile_pool(name="pav", bufs=1, space="PSUM"))  # attention out
        ptr = ctx.enter_context(tc.tile_pool(name="ptr", bufs=1, space="PSUM"))  # transposes

        # ---------------- constants / weights ----------------
        eye = wp.tile([128, 128], BF16)
        make_identity(nc, eye)
        ones1 = wp.tile([1, 128], BF16)
        nc.vector.memset(ones1, 1.0)
        # selector matrices for packing biases: sel_pk[16*(4*pk+i)+d, 32*i+d] = 1
        sels = []
        for pk in range(2):
            sel = wp.tile([128, 128], BF16, tag=f"sel{pk}")
            nc.vector.memset(sel.rearrange("p (b c) -> p b c", c=32)[:, :, D:32], 0.0)
            nc.vector.tensor_copy(
                out=sel.rearrange("p (b c) -> p b c", c=32)[:, :, 0:D],
                in_=eye[:, 64 * pk: 64 * pk + 64].rearrange("p (b c) -> p b c", c=D))
            sels.append(sel)

        _cast_engs = [nc.vector, nc.gpsimd, nc.scalar]
        _cast_i = [0]
        _dma_engs = [nc.sync]
        _dma_i = [0]

        def dma_rr(out, in_):
            eng = _dma_engs[_dma_i[0] % len(_dma_engs)]
            _dma_i[0] += 1
            eng.dma_start(out=out, in_=in_)

        def load_cast(name, src_ap, shape):
            """DMA f32 DRAM -> staging -> bf16 weight tile."""
            st = stg.tile(shape, F32, tag="stage")
            dma_rr(st, src_ap)
            wt = wp.tile(shape, BF16, tag=name)
            eng = _cast_engs[_cast_i[0] % 3]
            _cast_i[0] += 1
            if eng is nc.scalar:
                eng.copy(out=wt, in_=st)
            else:
                eng.tensor_copy(out=wt, in_=st)
            return wt

        # encoder weights: lhsT layout [K(part), M]
        w1 = load_cast("w1", ap["enc_W1"], [128, HID])                       # [128, 512]
        w2 = load_cast("w2", ap["enc_W2"].rearrange("(k p) m -> p k m", p=128), [128, KT, HID])
        qw = load_cast("qw", ap["q_W"].rearrange("(k p) m -> p k m", p=128), [128, 3 * KT, A])

        # per-partition biases, feature-major: [128, n_mtiles]
        def load_bias_fm(name, n_mt):
            bt = wp.tile([128, n_mt], F32, tag="b_" + name)
            dma_rr(bt, ap[name].rearrange("(m p) -> p m", p=128))
            return bt

        b1 = load_bias_fm("enc_b1", KT)
        b2 = load_bias_fm("enc_b2", KT)

        qb = wp.tile([1, A], BF16)
        qb_st = stg.tile([1, A], F32, tag="stage_s")
        dma_rr(qb_st, ap["q_b"].rearrange("(o a) -> o a", o=1))
        nc.gpsimd.tensor_copy(out=qb, in_=qb_st)

        layers = []
        for li in (1, 2):
            wv = load_cast(f"wv{li}", ap[f"Wv{li}"].rearrange("(k p) m -> p k m", p=128), [128, KT, HD])
            wo = load_cast(f"wo{li}", ap[f"Wo{li}"], [128, HID])
            bo = load_bias_fm(f"bo{li}", KT)
            bv = wp.tile([128, 1], F32, tag=f"bv{li}")
            dma_rr(bv, ap[f"bv{li}"].rearrange("(p o) -> p o", o=1))

            # packed q/k weights: pack pk holds heads pk*4+i at column band
            # 32*i..32*i+16. One natural-layout DMA per tensor; the packing is
            # a strided on-chip copy (cast included). Gap columns never feed
            # a matmul slice, so they are left unzeroed.
            packs = {}
            bnat = {}
            for nm in ("q", "k"):
                bn = stg.tile([128, 1], BF16, tag="bnat_" + nm)
                bn_f = stg.tile([128, 1], F32, tag="bnatf_" + nm)
                nc.sync.dma_start(out=bn_f, in_=ap[f"b{nm}{li}"].rearrange("(p o) -> p o", o=1))
                nc.vector.tensor_copy(out=bn, in_=bn_f)
                bnat[nm] = bn
            for nm in ("q", "k"):
                w_r = ap[f"W{nm}{li}"].rearrange("(k p) m -> p k m", p=128)
                stn = stg.tile([128, KT, 128], F32, tag="stage")
                nc.sync.dma_start(out=stn, in_=w_r)
                for pk in range(2):
                    wt = wp.tile([128, KT, 128], BF16, tag=f"w{nm}{li}{pk}")
                    nc.vector.memset(wt.rearrange("p k (b c) -> p k b c", c=32)[:, :, :, D:32], 0.0)
                    eng = _cast_engs[_cast_i[0] % 3]
                    _cast_i[0] += 1
                    dst = wt.rearrange("p k (b c) -> p k b c", c=32)[:, :, :, 0:D]
                    srcv = stn[:, :, 64 * pk: 64 * pk + 64].rearrange(
                        "p k (b c) -> p k b c", c=D)
                    if eng is nc.scalar:
                        eng.copy(out=dst, in_=srcv)
                    else:
                        eng.tensor_copy(out=dst, in_=srcv)
                    bt = wp.tile([128, 1], F32, tag=f"b{nm}{li}{pk}")
                    ps_b = ptr.tile([128, NT, 64], F32, tag="tr")
                    nc.tensor.matmul(ps_b[:, 0, 0:1], sels[pk], bnat[nm],
                                     start=True, stop=True)
                    nc.vector.tensor_copy(out=bt, in_=ps_b[:, 0, 0:1])
                    if nm == "q":
                        nc.scalar.mul(out=bt, in_=bt, mul=SCALE)
                    packs[(nm, pk)] = (wt, bt)
            layers.append(dict(wv=wv, bv=bv, wo=wo, bo=bo, packs=packs))

        # ---------------- per-pair program ----------------
        # Graphs are processed in PAIRS: every weight-stationary matmul
        # (encoder, q/k/v projections, output projection) uses a moving
        # operand that spans both graphs' nodes (N=512), so each LDWEIGHTS
        # is amortized over two graphs and instruction counts halve.
        # Attention itself (scores, exp, AV) stays per-graph.
        # Emitted as generators with yields at phase boundaries so pairs
        # interleave in each engine's FIFO (queues run in emission order).
        def pair_prog(gs):
            W = N * len(gs)          # moving-operand width for shared matmuls

            # ---- per-graph loads + mask/x prep ----
            mT_l, xq = [], []
            for g in gs:
                x_st = gio.tile([128, NT, F_IN], F32, tag="x")
                nc.sync.dma_start(out=x_st, in_=ap["x"][g].rearrange("(t p) f -> p t f", p=128))
                m_i = gio.tile([128, NT, N], I32, tag="mi")
                nc.sync.dma_start(out=m_i, in_=ap["mask"][g].rearrange("(t p) k -> p t k", p=128))
                m_b = sml.tile([128, NT, N], BF16, tag="mb", bufs=6)
                nc.gpsimd.tensor_copy(out=m_b, in_=m_i)
                mT = sml.tile([128, NT, N], BF16, tag="mT", bufs=6)
                for kt in range(NT):
                    ps = ptr.tile([128, NT, 128], BF16, tag="tr")
                    for qt in range(NT):
                        nc.tensor.transpose(ps[:, qt, :], m_b[:, qt, 128 * kt: 128 * (kt + 1)], eye)
                    nc.vector.tensor_copy(out=mT[:, kt, :].rearrange("p (t n) -> p t n", t=NT), in_=ps)
                mT_l.append(mT)
                xq.append((x_st, m_b))
            yield

            xT = sml.tile([128, len(gs), N], BF16, tag="xT")
            for gi, g in enumerate(gs):
                x_st, _ = xq[gi]
                x_b = sml.tile([128, NT, F_IN], BF16, tag="xb")
                nc.gpsimd.tensor_copy(out=x_b, in_=x_st)
                ps = ptr.tile([128, NT, 128], BF16, tag="tr")
                for t in range(NT):
                    nc.tensor.transpose(ps[:, t, :], x_b[:, t, :], eye)
                nc.vector.tensor_copy(out=xT[:, gi, :].rearrange("p (t n) -> p t n", t=NT), in_=ps)
            yield

            # ---- encoder (pair-wide N=W matmuls) ----
            h1 = sml.tile([128, KT, len(gs), N], BF16, tag="h1", bufs=3)
            for half in range(2):
                for j in range(2):
                    mt = half * 2 + j
                    ps = pmm.tile([128, len(gs), N], F32, tag="mm")
                    nc.tensor.matmul(ps.rearrange("p g n -> p (g n)"),
                                     w1[:, 128 * mt: 128 * (mt + 1)],
                                     xT.rearrange("p g n -> p (g n)"),
                                     start=True, stop=True)
                    nc.scalar.activation(out=h1[:, mt, :, :], in_=ps,
                                         func=mybir.ActivationFunctionType.Relu,
                                         bias=b1[:, mt: mt + 1], scale=1.0)
                yield
            h0 = act.tile([128, KT, len(gs), N], BF16, tag="h0")
            for half in range(2):
                for j in range(2):
                    mt = half * 2 + j
                    ps = pmm.tile([128, len(gs), N], F32, tag="mm")
                    for kt in range(KT):
                        nc.tensor.matmul(ps.rearrange("p g n -> p (g n)"),
                                         w2[:, kt, 128 * mt: 128 * (mt + 1)],
                                         h1[:, kt, :, :].rearrange("p g n -> p (g n)"),
                                         start=(kt == 0), stop=(kt == KT - 1))
                    nc.scalar.activation(out=h0[:, mt, :, :], in_=ps,
                                         func=mybir.ActivationFunctionType.Relu,
                                         bias=b2[:, mt: mt + 1], scale=1.0)
                yield

            # ---- attention layers ----
            h_in = h0
            h_keep = [h0]
            for li in range(2):
                L = layers[li]
                # q/k projections (packed, pair-wide)
                qkt = {}
                for nm in ("q", "k"):
                    out_t = sml.tile([128, 2, len(gs), N], BF16, tag=nm + "p")
                    for pk in range(2):
                        wt, bt = L["packs"][(nm, pk)]
                        ps = pmm.tile([128, len(gs), N], F32, tag="mm")
                        for kt in range(KT):
                            nc.tensor.matmul(ps.rearrange("p g n -> p (g n)"),
                                             wt[:, kt, :],
                                             h_in[:, kt, :, :].rearrange("p g n -> p (g n)"),
                                             start=(kt == 0), stop=(kt == KT - 1))
                        nc.scalar.activation(out=out_t[:, pk, :, :], in_=ps,
                                             func=mybir.ActivationFunctionType.Relu,
                                             bias=bt[:, 0:1],
                                             scale=SCALE if nm == "q" else 1.0)
                    qkt[nm] = out_t
                    yield
                qp, kp = qkt["q"], qkt["k"]

                # v projection (pair-wide), then per-graph v_ext
                ps_v = pmm.tile([128, len(gs), N], F32, tag="mm")
                for kt in range(KT):
                    nc.tensor.matmul(ps_v.rearrange("p g n -> p (g n)"),
                                     L["wv"][:, kt, :],
                                     h_in[:, kt, :, :].rearrange("p g n -> p (g n)"),
                                     start=(kt == 0), stop=(kt == KT - 1))
                vfm = sml.tile([128, len(gs), N], BF16, tag="vfm")
                nc.vector.tensor_scalar(out=vfm, in0=ps_v,
                                        scalar1=L["bv"][:, 0:1], scalar2=0.0,
                                        op0=mybir.AluOpType.add, op1=mybir.AluOpType.max)
                v_ext_l, v_ext_r_l = [], []
                for gi in range(len(gs)):
                    v_ext = sml.tile([128, NT, 17 * H], BF16, tag="vext")
                    ps = ptr.tile([128, NT, 128], BF16, tag="tr")
                    for t in range(NT):
                        nc.tensor.transpose(ps[:, t, :], vfm[:, gi, 128 * t: 128 * (t + 1)], eye)
                    v_ext_r = v_ext.rearrange("p t (h c) -> p t h c", c=17)
                    nc.vector.tensor_copy(out=v_ext_r[:, :, :, 0:D],
                                          in_=ps.rearrange("p t (h c) -> p t h c", c=D))
                    nc.vector.memset(v_ext_r[:, :, :, D:17], 1.0)
                    v_ext_l.append(v_ext)
                    v_ext_r_l.append(v_ext_r)
                yield

                # scores + exp + masked delta, per graph, heads in pairs.
                # Consecutive matmuls alternate 32-row bands (distinct PE row
                # groups + distinct psum banks) so weight loads can overlap
                # the previous matmul.
                me_l = [[] for _ in gs]
                for hq in range(H // 4):
                    heads = tuple(range(4 * hq, 4 * hq + 4))
                    for gi in range(len(gs)):
                        ps4 = psc.tile([128, 4, NT, N], F32, tag="sc")
                        for kt in range(NT):
                            for ix, hh in enumerate(heads):
                                pk, band = hh // 4, 32 * (hh % 4)
                                nc.tensor.matmul(ps4[:, ix, kt, :],
                                                 kp[band: band + D, pk, gi, 128 * kt: 128 * (kt + 1)],
                                                 qp[band: band + D, pk, gi, :],
                                                 start=(kt == 0), stop=(kt == NT - 1),
                                                 tile_position=(band, 0))
                        # one exp + one masked-exp for the head quad
                        e_s = esp.tile([128, 4, NT, N], BF16, tag="es")
                        nc.scalar.activation(out=e_s, in_=ps4,
                                             func=mybir.ActivationFunctionType.Exp)
                        me4 = mep.tile([128, 4, NT, N], BF16, tag="me")
                        mT = mT_l[gi]
                        mT_b = bass.AP(tensor=mT.tensor, offset=mT.offset,
                                       ap=[mT.ap[0], [0, 4], mT.ap[1], mT.ap[2]])
                        nc.vector.scalar_tensor_tensor(out=me4, in0=e_s, scalar=-1.0,
                                                       in1=mT_b,
                                                       op0=mybir.AluOpType.add,
                                                       op1=mybir.AluOpType.mult)
                        for ix in range(4):
                            me_l[gi].append(me4[:, ix, :, :])
                    yield

                # AV per graph: base + per-head deltas; one accumulation
                # group per psum bank (start on first, stop on last).
                ps_o_l = []
                for gi in range(len(gs)):
                    mT = mT_l[gi]
                    v_ext = v_ext_l[gi]
                    ps_o = pav.tile([128, NT, 17 * H], F32, tag="oext")
                    first = True
                    for qt in range(NT):
                        for kt in range(NT):
                            nc.tensor.matmul(ps_o[:, qt, :], mT[:, kt, 128 * qt: 128 * (qt + 1)],
                                             v_ext[:, kt, :], start=first, stop=False)
                            first = False
                    for hh in range(H):
                        me = me_l[gi][hh]
                        for qt in range(NT):
                            for kt in range(NT):
                                nc.tensor.matmul(ps_o[:, qt, 17 * hh: 17 * hh + 17],
                                                 me[:, kt, 128 * qt: 128 * (qt + 1)],
                                                 v_ext[:, kt, 17 * hh: 17 * hh + 17],
                                                 start=False,
                                                 stop=(hh == H - 1 and qt == NT - 1
                                                       and kt == NT - 1))
                    ps_o_l.append(ps_o)
                    yield

                # normalize + residual + transpose -> attT (both graphs)
                attT = sml.tile([128, len(gs), N], BF16, tag="attT")
                for gi in range(len(gs)):
                    ps_o_r = ps_o_l[gi].rearrange("p t (h c) -> p t h c", c=17)
                    att = sml.tile([128, NT, HD], BF16, tag="att")
                    rden = sml.tile([128, NT, H], F32, tag="rden")
                    nc.vector.reciprocal(out=rden, in_=ps_o_r[:, :, :, 16])
                    den_b = sml.tile([128, NT, H, D], BF16, tag="denb")
                    rden_bc = bass.AP(tensor=rden.tensor, offset=rden.offset,
                                      ap=[rden.ap[0], rden.ap[1], rden.ap[2], [0, D]])
                    nc.vector.tensor_copy(out=den_b, in_=rden_bc)
                    att_r = att.rearrange("p t (h c) -> p t h c", c=D)
                    nc.vector.tensor_mul(out=att_r, in0=ps_o_r[:, :, :, 0:D],
                                         in1=den_b)
                    nc.vector.tensor_add(out=att_r, in0=att_r,
                                         in1=v_ext_r_l[gi][:, :, :, 0:D])
                    ps = ptr.tile([128, NT, 128], BF16, tag="tr")
                    for qt in range(NT):
                        nc.tensor.transpose(ps[:, qt, :], att[:, qt, :], eye)
                    nc.vector.tensor_copy(out=attT[:, gi, :].rearrange("p (t n) -> p t n", t=NT), in_=ps)
                    yield

                # output projection (pair-wide)
                h_out = act.tile([128, KT, len(gs), N], BF16, tag=f"hL{li}")
                for half in range(2):
                    for j in range(2):
                        mt = half * 2 + j
                        ps2 = pmm.tile([128, len(gs), N], F32, tag="mm")
                        nc.tensor.matmul(ps2.rearrange("p g n -> p (g n)"),
                                         L["wo"][:, 128 * mt: 128 * (mt + 1)],
                                         attT.rearrange("p g n -> p (g n)"),
                                         start=True, stop=True)
                        nc.scalar.activation(out=h_out[:, mt, :, :], in_=ps2,
                                             func=mybir.ActivationFunctionType.Relu,
                                             bias=L["bo"][:, mt: mt + 1], scale=1.0)
                    yield
                h_keep.append(h_out)
                h_in = h_out

            # ---- final Q head (per graph; LDWEIGHTS here is tiny) ----
            for gi, g in enumerate(gs):
                ps_f = ptr.tile([128, NT, A], F32, tag="tr")
                for qt in range(NT):
                    nc.tensor.matmul(ps_f[:, qt, :], ones1, qb, start=True, stop=False)
                    for j in range(3):
                        src_t = h_keep[j]
                        for kt in range(KT):
                            nc.tensor.matmul(ps_f[:, qt, :],
                                             src_t[:, kt, gi, 128 * qt: 128 * (qt + 1)],
                                             qw[:, j * KT + kt, :],
                                             start=False,
                                             stop=(j == 2 and kt == KT - 1))
                o_sb = sml.tile([128, NT, A], F32, tag="osb")
                nc.vector.tensor_copy(out=o_sb, in_=ps_f)
                nc.sync.dma_start(out=ap["out"][g].rearrange("(t p) a -> p t a", p=128), in_=o_sb)
                yield

        # Drive the pair generators PIPE at a time, round-robin by phase,
        # with staggered starts so active pairs sit in different phases.
        PIPE = 3
        STAGGER = 5
        pairs = [list(range(i, min(i + 2, g_count))) for i in range(0, g_count, 2)]
        active = [pair_prog(pairs.pop(0))]
        rounds = 0
        while pairs or active:
            rounds += 1
            if rounds % STAGGER == 0 and len(active) < PIPE and pairs:
                active.append(pair_prog(pairs.pop(0)))
            for gen in list(active):
                try:
                    next(gen)
                except StopIteration:
                    active.remove(gen)
                    if pairs:
                        active.append(pair_prog(pairs.pop(0)))


def build(g_count=G, num_devices=NCORES):
    nc = bacc.Bacc("TRN2", target_bir_lowering=False, debug=False,
                   num_devices=num_devices)
    ap = {}
    ap["x"] = nc.dram_tensor("x", [g_count, N, F_IN], F32, kind="ExternalInput").ap()
    ap["mask"] = nc.dram_tensor("mask", [g_count, N, N], I32, kind="ExternalInput").ap()
    shapes = {
        "enc_W1": [F_IN, HID], "enc_b1": [HID], "enc_W2": [HID, HID], "enc_b2": [HID],
        "q_W": [3 * HID, A], "q_b": [A],
    }
    for li in (1, 2):
        shapes[f"Wv{li}"] = [HID, HD]; shapes[f"bv{li}"] = [HD]
        shapes[f"Wk{li}"] = [HID, HD]; shapes[f"bk{li}"] = [HD]
        shapes[f"Wq{li}"] = [HID, HD]; shapes[f"bq{li}"] = [HD]
        shapes[f"Wo{li}"] = [HD, HID]; shapes[f"bo{li}"] = [HID]
    for nm in WEIGHT_NAMES:
        ap[nm] = nc.dram_tensor(nm, shapes[nm], F32, kind="ExternalInput").ap()
    ap["out"] = nc.dram_tensor("out", [g_count, N, A], F32, kind="ExternalOutput").ap()

    with tile.TileContext(nc) as tc:
        _emit(nc, tc, ap, g_count)
    nc.compile()
    return nc


_NC_CACHE = {}


def kernel(**inputs):
    key = "full"
    if key not in _NC_CACHE:
        _NC_CACHE[key] = build(G, NCORES)
    nc = _NC_CACHE[key]

    from concourse import bass_utils
    in_maps = []
    for c in range(NCORES):
        m = {
            "x": np.ascontiguousarray(inputs["x"][c * G:(c + 1) * G], dtype=np.float32),
            "mask": np.ascontiguousarray(inputs["mask"][c * G:(c + 1) * G], dtype=np.int32),
        }
        for nm in WEIGHT_NAMES:
            m[nm] = np.ascontiguousarray(inputs[nm], dtype=np.float32)
        in_maps.append(m)
    res = bass_utils.run_bass_kernel_spmd(nc, in_maps, core_ids=list(range(NCORES)))
    return np.concatenate([r["out"] for r in res.results], axis=0)

